# revision 9
# baseline (speedup 1.0000x reference)
"""Trainium2 Bass kernel for ClassicPINN forward pass (15-layer tiny MLP, tanh).

Strategy
--------
Pure data parallel over 8 NeuronCores (131072 points each). Within a core,
points are processed in 4 chunks of 32768. Activations live feature-on-
partition: the 128 SBUF partitions hold G groups of the layer width, each
group handling a different 512-point column block. Layer weights are
block-diagonalized on the host (G copies of the tiny W^T along the
diagonal) so every matmul is a dense [K<=128, M<=128] x [K, 512] -> PSUM.
Four matmuls fill a 4-bank PSUM tile [128, 2048]; one ScalarE ACTIVATE
per PSUM tile applies tanh(x + b) (bias as a per-partition AP) and writes
SBUF, amortizing the ~352-cycle ACT overhead. ACT is the roofline here
(~216 tanh elements/point); PE, DMA and DVE all hide under it.

The same schedule object drives the Bass builder, a numpy simulator
(used by test.py), and an integer "point id" replay that yields the
output unpack permutation.
"""

import numpy as np
from contextlib import ExitStack

WIDTHS = [3, 8, 8, 8, 8, 8, 8, 8, 16, 16, 16, 32, 32, 32, 16, 3]
N_LAYERS = 15
N_POINTS = 1048576
NCORES = 8
PPC = N_POINTS // NCORES          # 131072 points per core
NCHUNKS = 4
CHUNK = PPC // NCHUNKS            # 32768 points per chunk
FREE = 512                        # matmul moving free dim (fp32 max)
ACT_BANKS = 4                     # PSUM banks per ACTIVATE (4*512 cols)


class _Layer:
    pass


def _make_schedule():
    """Per-layer matmul/activation schedule for one 32768-point chunk."""
    layers = []
    cur_groups = 16               # coords: 16 groups of 3 features = 48 partitions
    w_off = 0
    for i in range(N_LAYERS):
        in_w, out_w = WIDTHS[i], WIDTHS[i + 1]
        # Pad final width 3 -> 8 so Mmm=64 and PSUM packing stays on base
        # partitions {0, 64} (hardware rejects base partition 96).
        out_w_pad = 8 if out_w == 3 else out_w
        L = _Layer()
        L.i = i
        L.in_w, L.out_w, L.out_w_pad = in_w, out_w, out_w_pad
        L.Gmm = min(cur_groups, 128 // in_w, 128 // out_w_pad)
        L.Kmm = L.Gmm * in_w
        L.Mmm = L.Gmm * out_w_pad
        L.n_half = cur_groups // L.Gmm           # input partition slices
        L.pack = 128 // L.Mmm if L.Mmm in (32, 64) else 1
        L.in_groups = cur_groups
        L.in_cols = CHUNK // cur_groups
        ncb_in = L.in_cols // FREE
        L.n_mms = ncb_in * L.n_half
        L.out_groups = L.Gmm * L.pack
        L.out_cols = CHUNK // L.out_groups
        L.n_ocb = L.n_mms // L.pack              # 512-col output blocks
        L.n_psum = (L.n_ocb + ACT_BANKS - 1) // ACT_BANKS
        L.w_off = w_off
        w_off += L.Mmm
        L.mms = []
        for h in range(L.n_half):
            for cb in range(ncb_in):
                m = h * ncb_in + cb
                L.mms.append(dict(
                    rhs_p0=h * L.Kmm,
                    rhs_c0=cb * FREE,
                    out_p0=(m % L.pack) * L.Mmm,
                    ocb=m // L.pack,
                ))
        layers.append(L)
        cur_groups = L.out_groups
    return layers, w_off


_LAYERS, W_TOTAL = _make_schedule()


# ---------------------------------------------------------------- host packing

def pack_coords(coords):
    """[N_POINTS, 3] -> [NCORES, NCHUNKS, 48, 2048] matching the L0 layout.

    Per core: point n = chunk*32768 + t*8192 + g*512 + j lives at
    partition g*3+f, column t*512+j of tile [core, chunk].
    """
    c = np.ascontiguousarray(coords, dtype=np.float32)
    c = c.reshape(NCORES, NCHUNKS, 4, 16, FREE, 3)
    c = c.transpose(0, 1, 3, 5, 2, 4)            # core, chunk, g, f, t, j
    return np.ascontiguousarray(c.reshape(NCORES, NCHUNKS, 48, 2048))


def build_weights(Ws, bs):
    """Block-diagonal lhsT stack [128, W_TOTAL] and bias matrix [128, 15]."""
    lhsT_all = np.zeros((128, W_TOTAL), np.float32)
    biases = np.zeros((128, N_LAYERS), np.float32)
    for L in _LAYERS:
        W = np.asarray(Ws[L.i], np.float32)      # [out_w, in_w]
        bd = np.zeros((L.Kmm, L.Mmm), np.float32)
        for g in range(L.Gmm):
            bd[g * L.in_w:(g + 1) * L.in_w,
               g * L.out_w_pad:g * L.out_w_pad + L.out_w] = W.T
        for h in range(L.n_half):
            lhsT_all[h * L.Kmm:(h + 1) * L.Kmm, L.w_off:L.w_off + L.Mmm] = bd
        b = np.asarray(bs[L.i], np.float32)
        q = np.arange(128) % L.out_w_pad
        col = np.where(q < L.out_w, b[np.minimum(q, L.out_w - 1)], 0.0)
        biases[:, L.i] = col
    return lhsT_all, biases


def replay_ids():
    """Propagate chunk-local point ids through the schedule.

    Returns [128, out_cols] int array: element (p, c) of the final output
    tile holds component (p % out_w_pad) of chunk-local point ids[p, c].
    """
    ids = np.zeros((48, 2048), np.int64)
    j = np.arange(FREE)
    for g in range(16):
        for t in range(4):
            for f in range(3):
                ids[g * 3 + f, t * FREE:(t + 1) * FREE] = t * 8192 + g * FREE + j
    for L in _LAYERS:
        out = np.zeros((128, L.out_cols), np.int64)
        for mm in L.mms:
            src = ids[mm['rhs_p0']:mm['rhs_p0'] + L.Kmm:L.in_w,
                      mm['rhs_c0']:mm['rhs_c0'] + FREE]        # [Gmm, 512]
            out[mm['out_p0']:mm['out_p0'] + L.Mmm,
                mm['ocb'] * FREE:(mm['ocb'] + 1) * FREE] = \
                np.repeat(src, L.out_w_pad, axis=0)
        ids = out
    return ids


def simulate_chunk(coords_tile, lhsT_all, biases):
    """Numpy mirror of the device program for one [48, 2048] chunk tile."""
    act = coords_tile.astype(np.float32)
    for L in _LAYERS:
        out = np.zeros((128, L.out_cols), np.float32)
        for mm in L.mms:
            lhsT = lhsT_all[mm['rhs_p0']:mm['rhs_p0'] + L.Kmm,
                            L.w_off:L.w_off + L.Mmm]
            rhs = act[mm['rhs_p0']:mm['rhs_p0'] + L.Kmm,
                      mm['rhs_c0']:mm['rhs_c0'] + FREE]
            out[mm['out_p0']:mm['out_p0'] + L.Mmm,
                mm['ocb'] * FREE:(mm['ocb'] + 1) * FREE] = lhsT.T @ rhs
        out += biases[:, L.i:L.i + 1]
        act = np.tanh(out) if L.i < N_LAYERS - 1 else out
    return act                                   # [128, out_cols]


def unpack_output(per_core_out):
    """[NCORES][NCHUNKS, 128, out_cols] device tiles -> [N_POINTS, 3]."""
    ids = replay_ids()
    rows = np.arange(128)
    comp = rows % _LAYERS[-1].out_w_pad
    valid = comp < 3
    n_idx = ids[valid]
    o_idx = np.broadcast_to(comp[valid][:, None], n_idx.shape)
    out = np.empty((N_POINTS, 3), np.float32)
    for core in range(NCORES):
        tiles = per_core_out[core]
        for chunk in range(NCHUNKS):
            base = core * PPC + chunk * CHUNK
            out[base + n_idx, o_idx] = tiles[chunk][valid]
    return out


# ---------------------------------------------------------------- bass program

_PROGRAM_CACHE = {}


def _build_program():
    import concourse.bacc as bacc
    import concourse.bass as bass
    import concourse.tile as tile
    from concourse import mybir

    nc = bacc.Bacc("TRN2", target_bir_lowering=False, debug=False,
                   enable_asserts=False, num_devices=NCORES)
    dt = mybir.dt.float32
    coords_d = nc.dram_tensor("coords", (NCHUNKS, 48, 2048), dt,
                              kind="ExternalInput").ap()
    w_d = nc.dram_tensor("lhsT_all", (128, W_TOTAL), dt,
                         kind="ExternalInput").ap()
    b_d = nc.dram_tensor("biases", (128, N_LAYERS), dt,
                         kind="ExternalInput").ap()
    out_d = nc.dram_tensor("out", (NCHUNKS, 128, _LAYERS[-1].out_cols), dt,
                           kind="ExternalOutput").ap()

    TANH = mybir.ActivationFunctionType.Tanh
    IDENT = mybir.ActivationFunctionType.Identity

    with tile.TileContext(nc) as tc, ExitStack() as ctx:
        wpool = ctx.enter_context(tc.tile_pool(name="weights", bufs=1))
        cpool = ctx.enter_context(tc.tile_pool(name="cin", bufs=2))
        a8 = ctx.enter_context(tc.tile_pool(name="a8", bufs=3))
        a16 = ctx.enter_context(tc.tile_pool(name="a16", bufs=3))
        a32 = ctx.enter_context(tc.tile_pool(name="a32", bufs=2))
        pspool = ctx.enter_context(
            tc.tile_pool(name="psum", bufs=2, space="PSUM"))

        wt = wpool.tile([128, W_TOTAL], dt, tag="wt")
        nc.sync.dma_start(out=wt[:], in_=w_d[:])
        bt = wpool.tile([128, N_LAYERS], dt, tag="bt")
        nc.sync.dma_start(out=bt[:], in_=b_d[:])

        pool_by_cols = {2048: a8, 4096: a16, 8192: a32}

        for chunk in range(NCHUNKS):
            ct = cpool.tile([48, 2048], dt, tag="cin")
            nc.sync.dma_start(out=ct[:], in_=coords_d[chunk])
            act = ct
            for L in _LAYERS:
                pool = pool_by_cols[L.out_cols]
                out_t = pool.tile([128, L.out_cols], dt, tag=pool.name)
                for t in range(L.n_psum):
                    banks = min(ACT_BANKS, L.n_ocb - ACT_BANKS * t)
                    ps = pspool.tile([128, banks * FREE], dt, tag="ps")
                    for mm in L.mms[t * ACT_BANKS * L.pack:
                                    (t * ACT_BANKS + banks) * L.pack]:
                        lc = (mm['ocb'] - ACT_BANKS * t) * FREE
                        nc.tensor.matmul(
                            ps[mm['out_p0']:mm['out_p0'] + L.Mmm,
                               lc:lc + FREE],
                            wt[mm['rhs_p0']:mm['rhs_p0'] + L.Kmm,
                               L.w_off:L.w_off + L.Mmm],
                            act[mm['rhs_p0']:mm['rhs_p0'] + L.Kmm,
                                mm['rhs_c0']:mm['rhs_c0'] + FREE],
                            start=True, stop=True)
                    func = TANH if L.i < N_LAYERS - 1 else IDENT
                    c0 = t * ACT_BANKS * FREE
                    nc.scalar.activation(
                        out_t[:, c0:c0 + banks * FREE], ps[:],
                        func, bias=bt[:, L.i:L.i + 1])
                act = out_t
            nc.sync.dma_start(out=out_d[chunk], in_=act[:])

    nc.compile()
    return nc


def get_program():
    if "nc" not in _PROGRAM_CACHE:
        _PROGRAM_CACHE["nc"] = _build_program()
    return _PROGRAM_CACHE["nc"]


def make_in_maps(coords, Ws, bs):
    cp = pack_coords(coords)
    lhsT_all, biases = build_weights(Ws, bs)
    return [{"coords": cp[core], "lhsT_all": lhsT_all, "biases": biases}
            for core in range(NCORES)]


def kernel(**inputs):
    from concourse.bass_utils import run_bass_kernel_spmd

    coords = np.asarray(inputs["coords"], np.float32)
    Ws = [np.asarray(inputs[f"W{i}"], np.float32) for i in range(N_LAYERS)]
    bs = [np.asarray(inputs[f"b{i}"], np.float32) for i in range(N_LAYERS)]

    nc = get_program()
    in_maps = make_in_maps(coords, Ws, bs)
    res = run_bass_kernel_spmd(nc, in_maps, list(range(NCORES)))
    per_core = [res.results[c]["out"] for c in range(NCORES)]
    full = unpack_output(per_core)
    return (full[:, 0:1], full[:, 1:2], full[:, 2:3])


# revision 17
# speedup vs baseline: 1.1818x; 1.1818x over previous
"""Trainium2 Bass kernel for ClassicPINN forward pass (15-layer tiny MLP, tanh).

Strategy
--------
Pure data parallel over 8 NeuronCores (131072 points each). Within a core,
points are processed in 4 chunks of 32768. Activations live feature-on-
partition: the 128 SBUF partitions hold G groups of the layer width, each
group handling a different 512-point column block. Layer weights are
block-diagonalized on the host (G copies of the tiny W^T along the
diagonal) so every matmul is a dense [K<=128, M<=128] x [K, 512] -> PSUM.
Four matmuls fill a 4-bank PSUM tile [128, 2048]; one ScalarE ACTIVATE
per PSUM tile applies tanh(x + b) (bias as a per-partition AP) and writes
SBUF, amortizing the ~352-cycle ACT overhead. ACT is the roofline here
(~216 tanh elements/point); PE, DMA and DVE all hide under it.

The same schedule object drives the Bass builder, a numpy simulator
(used by test.py), and an integer "point id" replay that yields the
output unpack permutation.
"""

import numpy as np
from contextlib import ExitStack

WIDTHS = [3, 8, 8, 8, 8, 8, 8, 8, 16, 16, 16, 32, 32, 32, 16, 3]
N_LAYERS = 15
N_POINTS = 1048576
NCORES = 8
PPC = N_POINTS // NCORES          # 131072 points per core
NCHUNKS = 4
CHUNK = PPC // NCHUNKS            # 32768 points per chunk
FREE = 512                        # matmul moving free dim (fp32 max)
ACT_BANKS = 4                     # PSUM banks per ACTIVATE (4*512 cols)
MM_DTYPE = "tf32"                 # "f32" (exact, 4 cyc/row) | "tf32" (1 cyc/row)


class _Layer:
    pass


def _make_schedule():
    """Per-layer matmul/activation schedule for one 32768-point chunk."""
    layers = []
    cur_groups = 16               # coords: 16 groups of 3 features = 48 partitions
    w_off = 0
    for i in range(N_LAYERS):
        in_w, out_w = WIDTHS[i], WIDTHS[i + 1]
        # Pad final width 3 -> 8 so Mmm=64 and PSUM packing stays on base
        # partitions {0, 64} (hardware rejects base partition 96).
        out_w_pad = 8 if out_w == 3 else out_w
        L = _Layer()
        L.i = i
        L.in_w, L.out_w, L.out_w_pad = in_w, out_w, out_w_pad
        L.Gmm = min(cur_groups, 128 // in_w, 128 // out_w_pad)
        L.Kmm = L.Gmm * in_w
        L.Mmm = L.Gmm * out_w_pad
        L.n_half = cur_groups // L.Gmm           # input partition slices
        L.pack = 128 // L.Mmm if L.Mmm in (32, 64) else 1
        L.in_groups = cur_groups
        L.in_cols = CHUNK // cur_groups
        ncb_in = L.in_cols // FREE
        L.n_mms = ncb_in * L.n_half
        L.out_groups = L.Gmm * L.pack
        L.out_cols = CHUNK // L.out_groups
        L.n_ocb = L.n_mms // L.pack              # 512-col output blocks
        L.n_psum = (L.n_ocb + ACT_BANKS - 1) // ACT_BANKS
        L.w_off = w_off
        w_off += L.Mmm
        L.mms = []
        for h in range(L.n_half):
            for cb in range(ncb_in):
                m = h * ncb_in + cb
                L.mms.append(dict(
                    rhs_p0=h * L.Kmm,
                    rhs_c0=cb * FREE,
                    out_p0=(m % L.pack) * L.Mmm,
                    ocb=m // L.pack,
                ))
        layers.append(L)
        cur_groups = L.out_groups
    return layers, w_off


_LAYERS, W_TOTAL = _make_schedule()


# ---------------------------------------------------------------- host packing

def pack_coords(coords):
    """[N_POINTS, 3] -> [NCORES, NCHUNKS, 48, 2048] matching the L0 layout.

    Per core: point n = chunk*32768 + t*8192 + g*512 + j lives at
    partition g*3+f, column t*512+j of tile [core, chunk].
    """
    c = np.ascontiguousarray(coords, dtype=np.float32)
    c = c.reshape(NCORES, NCHUNKS, 4, 16, FREE, 3)
    c = c.transpose(0, 1, 3, 5, 2, 4)            # core, chunk, g, f, t, j
    return np.ascontiguousarray(c.reshape(NCORES, NCHUNKS, 48, 2048))


def build_weights(Ws, bs):
    """Block-diagonal lhsT stack [128, W_TOTAL] and bias matrix [128, 15]."""
    lhsT_all = np.zeros((128, W_TOTAL), np.float32)
    biases = np.zeros((128, N_LAYERS), np.float32)
    for L in _LAYERS:
        W = np.asarray(Ws[L.i], np.float32)      # [out_w, in_w]
        bd = np.zeros((L.Kmm, L.Mmm), np.float32)
        for g in range(L.Gmm):
            bd[g * L.in_w:(g + 1) * L.in_w,
               g * L.out_w_pad:g * L.out_w_pad + L.out_w] = W.T
        for h in range(L.n_half):
            lhsT_all[h * L.Kmm:(h + 1) * L.Kmm, L.w_off:L.w_off + L.Mmm] = bd
        b = np.asarray(bs[L.i], np.float32)
        q = np.arange(128) % L.out_w_pad
        col = np.where(q < L.out_w, b[np.minimum(q, L.out_w - 1)], 0.0)
        biases[:, L.i] = col
    return lhsT_all, biases


def replay_ids():
    """Propagate chunk-local point ids through the schedule.

    Returns [128, out_cols] int array: element (p, c) of the final output
    tile holds component (p % out_w_pad) of chunk-local point ids[p, c].
    """
    ids = np.zeros((48, 2048), np.int64)
    j = np.arange(FREE)
    for g in range(16):
        for t in range(4):
            for f in range(3):
                ids[g * 3 + f, t * FREE:(t + 1) * FREE] = t * 8192 + g * FREE + j
    for L in _LAYERS:
        out = np.zeros((128, L.out_cols), np.int64)
        for mm in L.mms:
            src = ids[mm['rhs_p0']:mm['rhs_p0'] + L.Kmm:L.in_w,
                      mm['rhs_c0']:mm['rhs_c0'] + FREE]        # [Gmm, 512]
            out[mm['out_p0']:mm['out_p0'] + L.Mmm,
                mm['ocb'] * FREE:(mm['ocb'] + 1) * FREE] = \
                np.repeat(src, L.out_w_pad, axis=0)
        ids = out
    return ids


def simulate_chunk(coords_tile, lhsT_all, biases):
    """Numpy mirror of the device program for one [48, 2048] chunk tile."""
    act = coords_tile.astype(np.float32)
    for L in _LAYERS:
        out = np.zeros((128, L.out_cols), np.float32)
        for mm in L.mms:
            lhsT = lhsT_all[mm['rhs_p0']:mm['rhs_p0'] + L.Kmm,
                            L.w_off:L.w_off + L.Mmm]
            rhs = act[mm['rhs_p0']:mm['rhs_p0'] + L.Kmm,
                      mm['rhs_c0']:mm['rhs_c0'] + FREE]
            out[mm['out_p0']:mm['out_p0'] + L.Mmm,
                mm['ocb'] * FREE:(mm['ocb'] + 1) * FREE] = lhsT.T @ rhs
        out += biases[:, L.i:L.i + 1]
        act = np.tanh(out) if L.i < N_LAYERS - 1 else out
    return act                                   # [128, out_cols]


def unpack_output(per_core_out):
    """[NCORES][NCHUNKS, 128, out_cols] device tiles -> [N_POINTS, 3]."""
    ids = replay_ids()
    rows = np.arange(128)
    comp = rows % _LAYERS[-1].out_w_pad
    valid = comp < 3
    n_idx = ids[valid]
    o_idx = np.broadcast_to(comp[valid][:, None], n_idx.shape)
    out = np.empty((N_POINTS, 3), np.float32)
    for core in range(NCORES):
        tiles = per_core_out[core]
        for chunk in range(NCHUNKS):
            base = core * PPC + chunk * CHUNK
            out[base + n_idx, o_idx] = tiles[chunk][valid]
    return out


# ---------------------------------------------------------------- bass program

_PROGRAM_CACHE = {}


def _build_program():
    import concourse.bacc as bacc
    import concourse.bass as bass
    import concourse.tile as tile
    from concourse import mybir

    nc = bacc.Bacc("TRN2", target_bir_lowering=False, debug=False,
                   enable_asserts=False, num_devices=NCORES)
    dt = mybir.dt.float32
    # float32r is TF32 (10-bit mantissa) streamed at 1 cycle/row through the
    # PE vs 4 for exact fp32. The whole matmul dataflow (coords, weights,
    # activations) must be typed float32r so the BIR verifier sees rounded
    # producers; PSUM accumulation and the bias path stay fp32.
    mdt = mybir.dt.float32r if MM_DTYPE == "tf32" else dt
    coords_d = nc.dram_tensor("coords", (NCHUNKS, 48, 2048), mdt,
                              kind="ExternalInput").ap()
    w_d = nc.dram_tensor("lhsT_all", (128, W_TOTAL), mdt,
                         kind="ExternalInput").ap()
    b_d = nc.dram_tensor("biases", (128, N_LAYERS), dt,
                         kind="ExternalInput").ap()
    out_d = nc.dram_tensor("out", (NCHUNKS, 128, _LAYERS[-1].out_cols), dt,
                           kind="ExternalOutput").ap()

    TANH = mybir.ActivationFunctionType.Tanh
    IDENT = mybir.ActivationFunctionType.Identity

    with tile.TileContext(nc) as tc, ExitStack() as ctx:
        wpool = ctx.enter_context(tc.tile_pool(name="weights", bufs=1))
        cpool = ctx.enter_context(tc.tile_pool(name="cin", bufs=2))
        a8 = ctx.enter_context(tc.tile_pool(name="a8", bufs=3))
        a16 = ctx.enter_context(tc.tile_pool(name="a16", bufs=3))
        a32 = ctx.enter_context(tc.tile_pool(name="a32", bufs=2))
        pspool = ctx.enter_context(
            tc.tile_pool(name="psum", bufs=2, space="PSUM"))

        wt = wpool.tile([128, W_TOTAL], mdt, tag="wt")
        nc.sync.dma_start(out=wt[:], in_=w_d[:])
        bt = wpool.tile([128, N_LAYERS], dt, tag="bt")
        nc.sync.dma_start(out=bt[:], in_=b_d[:])

        pool_by_cols = {2048: a8, 4096: a16, 8192: a32}

        for chunk in range(NCHUNKS):
            ct = cpool.tile([48, 2048], mdt, tag="cin")
            nc.sync.dma_start(out=ct[:], in_=coords_d[chunk])
            act = ct
            for L in _LAYERS:
                pool = pool_by_cols[L.out_cols]
                is_last = L.i == N_LAYERS - 1
                out_t = pool.tile([128, L.out_cols], dt if is_last else mdt,
                                  tag=pool.name)
                for t in range(L.n_psum):
                    banks = min(ACT_BANKS, L.n_ocb - ACT_BANKS * t)
                    ps = pspool.tile([128, banks * FREE], dt, tag="ps")
                    for mm in L.mms[t * ACT_BANKS * L.pack:
                                    (t * ACT_BANKS + banks) * L.pack]:
                        lc = (mm['ocb'] - ACT_BANKS * t) * FREE
                        w_ap = wt[mm['rhs_p0']:mm['rhs_p0'] + L.Kmm,
                                  L.w_off:L.w_off + L.Mmm]
                        x_ap = act[mm['rhs_p0']:mm['rhs_p0'] + L.Kmm,
                                   mm['rhs_c0']:mm['rhs_c0'] + FREE]
                        if mm['out_p0'] != 0 and mdt != dt:
                            # fp32r requires dst base partition 0; run the
                            # partition-packed tail matmuls as exact fp32.
                            w_ap, x_ap = w_ap.bitcast(dt), x_ap.bitcast(dt)
                        nc.tensor.matmul(
                            ps[mm['out_p0']:mm['out_p0'] + L.Mmm,
                               lc:lc + FREE],
                            w_ap, x_ap, start=True, stop=True)
                    func = TANH if not is_last else IDENT
                    c0 = t * ACT_BANKS * FREE
                    nc.scalar.activation(
                        out_t[:, c0:c0 + banks * FREE], ps[:],
                        func, bias=bt[:, L.i:L.i + 1])
                act = out_t
            nc.sync.dma_start(out=out_d[chunk], in_=act[:])

    nc.compile()
    return nc


def get_program():
    if "nc" not in _PROGRAM_CACHE:
        _PROGRAM_CACHE["nc"] = _build_program()
    return _PROGRAM_CACHE["nc"]


def make_in_maps(coords, Ws, bs):
    cp = pack_coords(coords)
    lhsT_all, biases = build_weights(Ws, bs)
    return [{"coords": cp[core], "lhsT_all": lhsT_all, "biases": biases}
            for core in range(NCORES)]


def kernel(**inputs):
    from concourse.bass_utils import run_bass_kernel_spmd

    coords = np.asarray(inputs["coords"], np.float32)
    Ws = [np.asarray(inputs[f"W{i}"], np.float32) for i in range(N_LAYERS)]
    bs = [np.asarray(inputs[f"b{i}"], np.float32) for i in range(N_LAYERS)]

    nc = get_program()
    in_maps = make_in_maps(coords, Ws, bs)
    res = run_bass_kernel_spmd(nc, in_maps, list(range(NCORES)))
    per_core = [res.results[c]["out"] for c in range(NCORES)]
    full = unpack_output(per_core)
    return (full[:, 0:1], full[:, 1:2], full[:, 2:3])


# revision 21
# speedup vs baseline: 2.7313x; 2.3111x over previous
"""Trainium2 Bass kernel for ClassicPINN forward pass (15-layer tiny MLP, tanh).

Strategy
--------
Pure data parallel over 8 NeuronCores (131072 points each). Within a core,
points are processed in 4 chunks of 32768. Activations live feature-on-
partition: the 128 SBUF partitions hold G groups of the layer width, each
group handling a different 512-point column block. Layer weights are
block-diagonalized on the host (G copies of the tiny W^T along the
diagonal) so every matmul is a dense [K<=128, M<=128] x [K, 512] -> PSUM.
Four matmuls fill a 4-bank PSUM tile [128, 2048]; one ScalarE ACTIVATE
per PSUM tile applies tanh(x + b) (bias as a per-partition AP) and writes
SBUF, amortizing the ~352-cycle ACT overhead. ACT is the roofline here
(~216 tanh elements/point); PE, DMA and DVE all hide under it.

The same schedule object drives the Bass builder, a numpy simulator
(used by test.py), and an integer "point id" replay that yields the
output unpack permutation.
"""

import numpy as np
from contextlib import ExitStack

WIDTHS = [3, 8, 8, 8, 8, 8, 8, 8, 16, 16, 16, 32, 32, 32, 16, 3]
N_LAYERS = 15
N_POINTS = 1048576
NCORES = 8
PPC = N_POINTS // NCORES          # 131072 points per core
NCHUNKS = 4
CHUNK = PPC // NCHUNKS            # 32768 points per chunk
FREE = 512                        # matmul moving free dim (fp32 max)
ACT_BANKS = 4                     # PSUM banks per ACTIVATE (4*512 cols)
MM_DTYPE = "tf32"                 # "f32" (exact, 4 cyc/row) | "tf32" (1 cyc/row)


class _Layer:
    pass


def _make_schedule():
    """Per-layer matmul/activation schedule for one 32768-point chunk."""
    layers = []
    cur_groups = 16               # coords: 16 groups of 3 features = 48 partitions
    w_off = 0
    for i in range(N_LAYERS):
        in_w, out_w = WIDTHS[i], WIDTHS[i + 1]
        # Pad final width 3 -> 8 so Mmm=64 and PSUM packing stays on base
        # partitions {0, 64} (hardware rejects base partition 96).
        out_w_pad = 8 if out_w == 3 else out_w
        L = _Layer()
        L.i = i
        L.in_w, L.out_w, L.out_w_pad = in_w, out_w, out_w_pad
        L.Gmm = min(cur_groups, 128 // in_w, 128 // out_w_pad)
        L.Kmm = L.Gmm * in_w
        L.Mmm = L.Gmm * out_w_pad
        L.n_half = cur_groups // L.Gmm           # input partition slices
        L.pack = 128 // L.Mmm if L.Mmm in (32, 64) else 1
        L.in_groups = cur_groups
        L.in_cols = CHUNK // cur_groups
        ncb_in = L.in_cols // FREE
        L.n_mms = ncb_in * L.n_half
        L.out_groups = L.Gmm * L.pack
        L.out_cols = CHUNK // L.out_groups
        L.n_ocb = L.n_mms // L.pack              # 512-col output blocks
        L.n_psum = (L.n_ocb + ACT_BANKS - 1) // ACT_BANKS
        L.w_off = w_off
        w_off += L.Mmm
        L.mms = []
        for h in range(L.n_half):
            for cb in range(ncb_in):
                m = h * ncb_in + cb
                L.mms.append(dict(
                    rhs_p0=h * L.Kmm,
                    rhs_c0=cb * FREE,
                    out_p0=(m % L.pack) * L.Mmm,
                    ocb=m // L.pack,
                ))
        layers.append(L)
        cur_groups = L.out_groups
    return layers, w_off


_LAYERS, W_TOTAL = _make_schedule()


# ---------------------------------------------------------------- host packing

def pack_coords(coords):
    """[N_POINTS, 3] -> [NCORES, NCHUNKS, 48, 2048] matching the L0 layout.

    Per core: point n = chunk*32768 + t*8192 + g*512 + j lives at
    partition g*3+f, column t*512+j of tile [core, chunk].
    """
    c = np.ascontiguousarray(coords, dtype=np.float32)
    c = c.reshape(NCORES, NCHUNKS, 4, 16, FREE, 3)
    c = c.transpose(0, 1, 3, 5, 2, 4)            # core, chunk, g, f, t, j
    return np.ascontiguousarray(c.reshape(NCORES, NCHUNKS, 48, 2048))


def build_weights(Ws, bs):
    """Block-diagonal lhsT stack [128, W_TOTAL] and bias matrix [128, 15]."""
    lhsT_all = np.zeros((128, W_TOTAL), np.float32)
    biases = np.zeros((128, N_LAYERS), np.float32)
    for L in _LAYERS:
        W = np.asarray(Ws[L.i], np.float32)      # [out_w, in_w]
        bd = np.zeros((L.Kmm, L.Mmm), np.float32)
        for g in range(L.Gmm):
            bd[g * L.in_w:(g + 1) * L.in_w,
               g * L.out_w_pad:g * L.out_w_pad + L.out_w] = W.T
        for h in range(L.n_half):
            lhsT_all[h * L.Kmm:(h + 1) * L.Kmm, L.w_off:L.w_off + L.Mmm] = bd
        b = np.asarray(bs[L.i], np.float32)
        q = np.arange(128) % L.out_w_pad
        col = np.where(q < L.out_w, b[np.minimum(q, L.out_w - 1)], 0.0)
        biases[:, L.i] = col
    return lhsT_all, biases


def replay_ids():
    """Propagate chunk-local point ids through the schedule.

    Returns [128, out_cols] int array: element (p, c) of the final output
    tile holds component (p % out_w_pad) of chunk-local point ids[p, c].
    """
    ids = np.zeros((48, 2048), np.int64)
    j = np.arange(FREE)
    for g in range(16):
        for t in range(4):
            for f in range(3):
                ids[g * 3 + f, t * FREE:(t + 1) * FREE] = t * 8192 + g * FREE + j
    for L in _LAYERS:
        out = np.zeros((128, L.out_cols), np.int64)
        for mm in L.mms:
            src = ids[mm['rhs_p0']:mm['rhs_p0'] + L.Kmm:L.in_w,
                      mm['rhs_c0']:mm['rhs_c0'] + FREE]        # [Gmm, 512]
            out[mm['out_p0']:mm['out_p0'] + L.Mmm,
                mm['ocb'] * FREE:(mm['ocb'] + 1) * FREE] = \
                np.repeat(src, L.out_w_pad, axis=0)
        ids = out
    return ids


def simulate_chunk(coords_tile, lhsT_all, biases):
    """Numpy mirror of the device program for one [48, 2048] chunk tile."""
    act = coords_tile.astype(np.float32)
    for L in _LAYERS:
        out = np.zeros((128, L.out_cols), np.float32)
        for mm in L.mms:
            lhsT = lhsT_all[mm['rhs_p0']:mm['rhs_p0'] + L.Kmm,
                            L.w_off:L.w_off + L.Mmm]
            rhs = act[mm['rhs_p0']:mm['rhs_p0'] + L.Kmm,
                      mm['rhs_c0']:mm['rhs_c0'] + FREE]
            out[mm['out_p0']:mm['out_p0'] + L.Mmm,
                mm['ocb'] * FREE:(mm['ocb'] + 1) * FREE] = lhsT.T @ rhs
        out += biases[:, L.i:L.i + 1]
        act = np.tanh(out) if L.i < N_LAYERS - 1 else out
    return act                                   # [128, out_cols]


def unpack_output(per_core_out):
    """[NCORES][NCHUNKS, 128, out_cols] device tiles -> [N_POINTS, 3]."""
    ids = replay_ids()
    rows = np.arange(128)
    comp = rows % _LAYERS[-1].out_w_pad
    valid = comp < 3
    n_idx = ids[valid]
    o_idx = np.broadcast_to(comp[valid][:, None], n_idx.shape)
    out = np.empty((N_POINTS, 3), np.float32)
    for core in range(NCORES):
        tiles = per_core_out[core]
        for chunk in range(NCHUNKS):
            base = core * PPC + chunk * CHUNK
            out[base + n_idx, o_idx] = tiles[chunk][valid]
    return out


# ---------------------------------------------------------------- bass program

_PROGRAM_CACHE = {}


def _build_program(repeat=1):
    import concourse.bacc as bacc
    import concourse.bass as bass
    import concourse.tile as tile
    from concourse import mybir

    nc = bacc.Bacc("TRN2", target_bir_lowering=False, debug=False,
                   enable_asserts=False, num_devices=NCORES)
    dt = mybir.dt.float32
    # float32r is TF32 (10-bit mantissa) streamed at 1 cycle/row through the
    # PE vs 4 for exact fp32. The whole matmul dataflow (coords, weights,
    # activations) must be typed float32r so the BIR verifier sees rounded
    # producers; PSUM accumulation and the bias path stay fp32.
    mdt = mybir.dt.float32r if MM_DTYPE == "tf32" else dt
    coords_d = nc.dram_tensor("coords", (NCHUNKS, 48, 2048), mdt,
                              kind="ExternalInput").ap()
    w_d = nc.dram_tensor("lhsT_all", (128, W_TOTAL), mdt,
                         kind="ExternalInput").ap()
    b_d = nc.dram_tensor("biases", (128, N_LAYERS), dt,
                         kind="ExternalInput").ap()
    out_d = nc.dram_tensor("out", (NCHUNKS, 128, _LAYERS[-1].out_cols), dt,
                           kind="ExternalOutput").ap()

    TANH = mybir.ActivationFunctionType.Tanh
    IDENT = mybir.ActivationFunctionType.Identity

    with tile.TileContext(nc) as tc, ExitStack() as ctx:
        wpool = ctx.enter_context(tc.tile_pool(name="weights", bufs=1))
        cpool = ctx.enter_context(tc.tile_pool(name="cin", bufs=2))
        a8 = ctx.enter_context(tc.tile_pool(name="a8", bufs=3))
        a16 = ctx.enter_context(tc.tile_pool(name="a16", bufs=3))
        a32 = ctx.enter_context(tc.tile_pool(name="a32", bufs=2))
        pspool = ctx.enter_context(
            tc.tile_pool(name="psum", bufs=2, space="PSUM"))

        wt = wpool.tile([128, W_TOTAL], mdt, tag="wt")
        nc.sync.dma_start(out=wt[:], in_=w_d[:])
        bt = wpool.tile([128, N_LAYERS], dt, tag="bt")
        nc.sync.dma_start(out=bt[:], in_=b_d[:])

        pool_by_cols = {2048: a8, 4096: a16, 8192: a32}

        for chunk in [c for _ in range(repeat) for c in range(NCHUNKS)]:
            ct = cpool.tile([48, 2048], mdt, tag="cin")
            nc.sync.dma_start(out=ct[:], in_=coords_d[chunk])
            act = ct
            for L in _LAYERS:
                pool = pool_by_cols[L.out_cols]
                is_last = L.i == N_LAYERS - 1
                out_t = pool.tile([128, L.out_cols], dt if is_last else mdt,
                                  tag=pool.name)
                for t in range(L.n_psum):
                    banks = min(ACT_BANKS, L.n_ocb - ACT_BANKS * t)
                    ps = pspool.tile([128, banks * FREE], dt, tag="ps")
                    for mm in L.mms[t * ACT_BANKS * L.pack:
                                    (t * ACT_BANKS + banks) * L.pack]:
                        lc = (mm['ocb'] - ACT_BANKS * t) * FREE
                        w_ap = wt[mm['rhs_p0']:mm['rhs_p0'] + L.Kmm,
                                  L.w_off:L.w_off + L.Mmm]
                        x_ap = act[mm['rhs_p0']:mm['rhs_p0'] + L.Kmm,
                                   mm['rhs_c0']:mm['rhs_c0'] + FREE]
                        if mm['out_p0'] != 0 and mdt != dt:
                            # fp32r requires dst base partition 0; run the
                            # partition-packed tail matmuls as exact fp32.
                            w_ap, x_ap = w_ap.bitcast(dt), x_ap.bitcast(dt)
                        nc.tensor.matmul(
                            ps[mm['out_p0']:mm['out_p0'] + L.Mmm,
                               lc:lc + FREE],
                            w_ap, x_ap, start=True, stop=True)
                    c0 = t * ACT_BANKS * FREE
                    if is_last:
                        # Final layer: bias-add on the otherwise-idle DVE, so
                        # ACT never leaves the Tanh table set.
                        nc.vector.tensor_scalar_add(
                            out_t[:, c0:c0 + banks * FREE], ps[:],
                            bt[:, L.i:L.i + 1])
                    else:
                        nc.scalar.activation(
                            out_t[:, c0:c0 + banks * FREE], ps[:],
                            TANH, bias=bt[:, L.i:L.i + 1])
                act = out_t
            nc.sync.dma_start(out=out_d[chunk], in_=act[:])

    nc.compile()
    return nc


def get_program(repeat=1):
    key = ("nc", repeat)
    if key not in _PROGRAM_CACHE:
        _PROGRAM_CACHE[key] = _build_program(repeat)
    return _PROGRAM_CACHE[key]


def make_in_maps(coords, Ws, bs):
    cp = pack_coords(coords)
    lhsT_all, biases = build_weights(Ws, bs)
    return [{"coords": cp[core], "lhsT_all": lhsT_all, "biases": biases}
            for core in range(NCORES)]


def kernel(**inputs):
    from concourse.bass_utils import run_bass_kernel_spmd

    coords = np.asarray(inputs["coords"], np.float32)
    Ws = [np.asarray(inputs[f"W{i}"], np.float32) for i in range(N_LAYERS)]
    bs = [np.asarray(inputs[f"b{i}"], np.float32) for i in range(N_LAYERS)]

    nc = get_program()
    in_maps = make_in_maps(coords, Ws, bs)
    res = run_bass_kernel_spmd(nc, in_maps, list(range(NCORES)))
    per_core = [res.results[c]["out"] for c in range(NCORES)]
    full = unpack_output(per_core)
    return (full[:, 0:1], full[:, 1:2], full[:, 2:3])


# revision 29
# speedup vs baseline: 2.7517x; 1.0075x over previous
"""Trainium2 Bass kernel for ClassicPINN forward pass (15-layer tiny MLP, tanh).

Strategy
--------
Pure data parallel over 8 NeuronCores (131072 points each). Within a core,
points are processed in 4 chunks of 32768. Activations live feature-on-
partition: the 128 SBUF partitions hold G groups of the layer width, each
group handling a different 512-point column block. Layer weights are
block-diagonalized on the host (G copies of the tiny W^T along the
diagonal) so every matmul is a dense [K<=128, M<=128] x [K, 512] -> PSUM.
Four matmuls fill a 4-bank PSUM tile [128, 2048]; one ScalarE ACTIVATE
per PSUM tile applies tanh(x + b) (bias as a per-partition AP) and writes
SBUF, amortizing the ~352-cycle ACT overhead. ACT is the roofline here
(~216 tanh elements/point); PE, DMA and DVE all hide under it.

The same schedule object drives the Bass builder, a numpy simulator
(used by test.py), and an integer "point id" replay that yields the
output unpack permutation.
"""

import numpy as np
from contextlib import ExitStack

WIDTHS = [3, 8, 8, 8, 8, 8, 8, 8, 16, 16, 16, 32, 32, 32, 16, 3]
N_LAYERS = 15
N_POINTS = 1048576
NCORES = 8
PPC = N_POINTS // NCORES          # 131072 points per core
NCHUNKS = 4
CHUNK = PPC // NCHUNKS            # 32768 points per chunk
FREE = 512                        # matmul moving free dim (fp32 max)
ACT_BANKS = 4                     # PSUM banks per ACTIVATE (4*512 cols)
MM_DTYPE = "tf32"                 # "f32" (exact, 4 cyc/row) | "tf32" (1 cyc/row)


class _Layer:
    pass


def _make_schedule():
    """Per-layer matmul/activation schedule for one 32768-point chunk."""
    layers = []
    cur_groups = 16               # coords: 16 groups of 3 features = 48 partitions
    w_off = 0
    for i in range(N_LAYERS):
        in_w, out_w = WIDTHS[i], WIDTHS[i + 1]
        out_w_pad = 8 if out_w == 3 else out_w   # pad final width 3 -> 8
        L = _Layer()
        L.i = i
        L.in_w, L.out_w, L.out_w_pad = in_w, out_w, out_w_pad
        L.Gmm = min(cur_groups, 128 // in_w, 128 // out_w_pad)
        L.Kmm = L.Gmm * in_w
        L.Mmm = L.Gmm * out_w_pad
        L.n_half = cur_groups // L.Gmm           # input partition slices
        # fp32r matmuls must write PSUM base partition 0, so never stack
        # multiple matmuls partition-wise in a bank; short layers just
        # produce short (Mmm-high) tiles.
        L.pack = 1
        L.out_height = L.Mmm
        L.in_groups = cur_groups
        L.in_cols = CHUNK // cur_groups
        ncb_in = L.in_cols // FREE
        L.n_mms = ncb_in * L.n_half
        L.out_groups = L.Gmm
        L.out_cols = CHUNK // L.out_groups
        L.n_ocb = L.n_mms                        # 512-col output blocks
        L.n_psum = (L.n_ocb + ACT_BANKS - 1) // ACT_BANKS
        L.w_off = w_off
        w_off += L.Mmm
        L.mms = []
        for h in range(L.n_half):
            for cb in range(ncb_in):
                m = h * ncb_in + cb
                L.mms.append(dict(
                    rhs_p0=h * L.Kmm,
                    rhs_c0=cb * FREE,
                    out_p0=0,
                    ocb=m,
                ))
        layers.append(L)
        cur_groups = L.out_groups
    return layers, w_off


_LAYERS, W_TOTAL = _make_schedule()


# ---------------------------------------------------------------- host packing

def pack_coords(coords):
    """[N_POINTS, 3] -> [NCORES, NCHUNKS, 48, 2048] matching the L0 layout.

    Per core: point n = chunk*32768 + t*8192 + g*512 + j lives at
    partition g*3+f, column t*512+j of tile [core, chunk].
    """
    c = np.ascontiguousarray(coords, dtype=np.float32)
    c = c.reshape(NCORES, NCHUNKS, 4, 16, FREE, 3)
    c = c.transpose(0, 1, 3, 5, 2, 4)            # core, chunk, g, f, t, j
    return np.ascontiguousarray(c.reshape(NCORES, NCHUNKS, 48, 2048))


def build_weights(Ws, bs):
    """Block-diagonal lhsT stack [128, W_TOTAL] and bias matrix [128, 15]."""
    lhsT_all = np.zeros((128, W_TOTAL), np.float32)
    biases = np.zeros((128, N_LAYERS), np.float32)
    for L in _LAYERS:
        W = np.asarray(Ws[L.i], np.float32)      # [out_w, in_w]
        bd = np.zeros((L.Kmm, L.Mmm), np.float32)
        for g in range(L.Gmm):
            bd[g * L.in_w:(g + 1) * L.in_w,
               g * L.out_w_pad:g * L.out_w_pad + L.out_w] = W.T
        for h in range(L.n_half):
            lhsT_all[h * L.Kmm:(h + 1) * L.Kmm, L.w_off:L.w_off + L.Mmm] = bd
        b = np.asarray(bs[L.i], np.float32)
        q = np.arange(128) % L.out_w_pad
        col = np.where(q < L.out_w, b[np.minimum(q, L.out_w - 1)], 0.0)
        biases[:, L.i] = col
    return lhsT_all, biases


def replay_ids():
    """Propagate chunk-local point ids through the schedule.

    Returns [128, out_cols] int array: element (p, c) of the final output
    tile holds component (p % out_w_pad) of chunk-local point ids[p, c].
    """
    ids = np.zeros((48, 2048), np.int64)
    j = np.arange(FREE)
    for g in range(16):
        for t in range(4):
            for f in range(3):
                ids[g * 3 + f, t * FREE:(t + 1) * FREE] = t * 8192 + g * FREE + j
    for L in _LAYERS:
        out = np.zeros((L.out_height, L.out_cols), np.int64)
        for mm in L.mms:
            src = ids[mm['rhs_p0']:mm['rhs_p0'] + L.Kmm:L.in_w,
                      mm['rhs_c0']:mm['rhs_c0'] + FREE]        # [Gmm, 512]
            out[mm['out_p0']:mm['out_p0'] + L.Mmm,
                mm['ocb'] * FREE:(mm['ocb'] + 1) * FREE] = \
                np.repeat(src, L.out_w_pad, axis=0)
        ids = out
    return ids


def simulate_chunk(coords_tile, lhsT_all, biases):
    """Numpy mirror of the device program for one [48, 2048] chunk tile."""
    act = coords_tile.astype(np.float32)
    for L in _LAYERS:
        out = np.zeros((L.out_height, L.out_cols), np.float32)
        for mm in L.mms:
            lhsT = lhsT_all[mm['rhs_p0']:mm['rhs_p0'] + L.Kmm,
                            L.w_off:L.w_off + L.Mmm]
            rhs = act[mm['rhs_p0']:mm['rhs_p0'] + L.Kmm,
                      mm['rhs_c0']:mm['rhs_c0'] + FREE]
            out[mm['out_p0']:mm['out_p0'] + L.Mmm,
                mm['ocb'] * FREE:(mm['ocb'] + 1) * FREE] = lhsT.T @ rhs
        out += biases[:L.out_height, L.i:L.i + 1]
        act = np.tanh(out) if L.i < N_LAYERS - 1 else out
    return act                                   # [out_height, out_cols]


def unpack_output(per_core_out):
    """[NCORES][NCHUNKS, 128, out_cols] device tiles -> [N_POINTS, 3]."""
    ids = replay_ids()
    rows = np.arange(_LAYERS[-1].out_height)
    comp = rows % _LAYERS[-1].out_w_pad
    valid = comp < 3
    n_idx = ids[valid]
    o_idx = np.broadcast_to(comp[valid][:, None], n_idx.shape)
    out = np.empty((N_POINTS, 3), np.float32)
    for core in range(NCORES):
        tiles = per_core_out[core]
        for chunk in range(NCHUNKS):
            base = core * PPC + chunk * CHUNK
            out[base + n_idx, o_idx] = tiles[chunk][valid]
    return out


# ---------------------------------------------------------------- bass program

_PROGRAM_CACHE = {}


def _build_program(repeat=1):
    import concourse.bacc as bacc
    import concourse.bass as bass
    import concourse.tile as tile
    from concourse import mybir

    nc = bacc.Bacc("TRN2", target_bir_lowering=False, debug=False,
                   enable_asserts=False, num_devices=NCORES)
    dt = mybir.dt.float32
    # float32r is TF32 (10-bit mantissa) streamed at 1 cycle/row through the
    # PE vs 4 for exact fp32. The whole matmul dataflow (coords, weights,
    # activations) must be typed float32r so the BIR verifier sees rounded
    # producers; PSUM accumulation and the bias path stay fp32.
    mdt = mybir.dt.float32r if MM_DTYPE == "tf32" else dt
    coords_d = nc.dram_tensor("coords", (NCHUNKS, 48, 2048), mdt,
                              kind="ExternalInput").ap()
    w_d = nc.dram_tensor("lhsT_all", (128, W_TOTAL), mdt,
                         kind="ExternalInput").ap()
    b_d = nc.dram_tensor("biases", (128, N_LAYERS), dt,
                         kind="ExternalInput").ap()
    out_d = nc.dram_tensor(
        "out", (NCHUNKS, _LAYERS[-1].out_height, _LAYERS[-1].out_cols), dt,
        kind="ExternalOutput").ap()

    TANH = mybir.ActivationFunctionType.Tanh

    with tile.TileContext(nc) as tc, ExitStack() as ctx:
        wpool = ctx.enter_context(tc.tile_pool(name="weights", bufs=1))
        cpool = ctx.enter_context(tc.tile_pool(name="cin", bufs=2))
        a8 = ctx.enter_context(tc.tile_pool(name="a8", bufs=4))
        a16 = ctx.enter_context(tc.tile_pool(name="a16", bufs=4))
        a32 = ctx.enter_context(tc.tile_pool(name="a32", bufs=2))
        pspool = ctx.enter_context(
            tc.tile_pool(name="psum", bufs=2, space="PSUM"))

        wt = wpool.tile([128, W_TOTAL], mdt, tag="wt")
        nc.sync.dma_start(out=wt[:], in_=w_d[:])
        bt = wpool.tile([128, N_LAYERS], dt, tag="bt")
        nc.sync.dma_start(out=bt[:], in_=b_d[:])

        pool_by_cols = {2048: a8, 4096: a16, 8192: a32}

        def emit_layer(L, act):
            pool = pool_by_cols[L.out_cols]
            is_last = L.i == N_LAYERS - 1
            H = L.out_height
            out_t = pool.tile([H, L.out_cols], dt if is_last else mdt,
                              tag=pool.name)
            for t in range(L.n_psum):
                banks = min(ACT_BANKS, L.n_ocb - ACT_BANKS * t)
                ps = pspool.tile([H, banks * FREE], dt, tag="ps")
                for mm in L.mms[t * ACT_BANKS:t * ACT_BANKS + banks]:
                    lc = (mm['ocb'] - ACT_BANKS * t) * FREE
                    nc.tensor.matmul(
                        ps[0:L.Mmm, lc:lc + FREE],
                        wt[mm['rhs_p0']:mm['rhs_p0'] + L.Kmm,
                           L.w_off:L.w_off + L.Mmm],
                        act[mm['rhs_p0']:mm['rhs_p0'] + L.Kmm,
                            mm['rhs_c0']:mm['rhs_c0'] + FREE],
                        start=True, stop=True)
                c0 = t * ACT_BANKS * FREE
                if is_last:
                    # Final layer: bias-add on the otherwise-idle DVE, so
                    # ACT never leaves the Tanh table set.
                    nc.vector.tensor_scalar_add(
                        out_t[:, c0:c0 + banks * FREE], ps[:],
                        bt[0:H, L.i:L.i + 1])
                else:
                    nc.scalar.activation(
                        out_t[:, c0:c0 + banks * FREE], ps[:],
                        TANH, bias=bt[0:H, L.i:L.i + 1])
            return out_t

        # Chunks run in interleaved pairs through the narrow (1-2 ACTs per
        # layer) early layers so ACT ping-pongs between the two chunks'
        # PSUM tiles; the wide layers (>=4 PSUM tiles each) self-pipeline
        # and run per-chunk to fit SBUF.
        seq = [c for _ in range(repeat) for c in range(NCHUNKS)]
        N_PAIRED = 10
        for ca, cb in zip(seq[0::2], seq[1::2]):
            acts = {}
            for c in (ca, cb):
                ct = cpool.tile([48, 2048], mdt, tag="cin")
                nc.sync.dma_start(out=ct[:], in_=coords_d[c])
                acts[c] = ct
            for L in _LAYERS[:N_PAIRED]:
                for c in (ca, cb):
                    acts[c] = emit_layer(L, acts[c])
            for c in (ca, cb):
                act = acts[c]
                for L in _LAYERS[N_PAIRED:]:
                    act = emit_layer(L, act)
                nc.sync.dma_start(out=out_d[c], in_=act[:])

    nc.compile()
    return nc


def get_program(repeat=1):
    key = ("nc", repeat)
    if key not in _PROGRAM_CACHE:
        _PROGRAM_CACHE[key] = _build_program(repeat)
    return _PROGRAM_CACHE[key]


def make_in_maps(coords, Ws, bs):
    cp = pack_coords(coords)
    lhsT_all, biases = build_weights(Ws, bs)
    return [{"coords": cp[core], "lhsT_all": lhsT_all, "biases": biases}
            for core in range(NCORES)]


def kernel(**inputs):
    from concourse.bass_utils import run_bass_kernel_spmd

    coords = np.asarray(inputs["coords"], np.float32)
    Ws = [np.asarray(inputs[f"W{i}"], np.float32) for i in range(N_LAYERS)]
    bs = [np.asarray(inputs[f"b{i}"], np.float32) for i in range(N_LAYERS)]

    nc = get_program()
    in_maps = make_in_maps(coords, Ws, bs)
    res = run_bass_kernel_spmd(nc, in_maps, list(range(NCORES)))
    per_core = [res.results[c]["out"] for c in range(NCORES)]
    full = unpack_output(per_core)
    return (full[:, 0:1], full[:, 1:2], full[:, 2:3])
